# revision 10
# baseline (speedup 1.0000x reference)
"""CCRNN (gnn_message_passing) Trainium2 Bass kernel.

Data-parallel over batch B=16 across 8 NeuronCores (2 per core). The whole
recurrence runs SBUF-resident. Two on-chip layouts are maintained:
  node layout    [n_chunk][128 nodes, (b, d) free]   -> graph-diffusion matmuls
  feature layout [d features (partitions), (b, n)]   -> feature matmuls / gates
Graph diffusion (Chebyshev) uses two matmul forms, both consuming node layout:
  form-1: out[m,(b,d)] = sum_n ST[n,m]^T . x[n,(b,d)]      (node -> node)
  form-2: out[d,m]     = sum_n x[n,d]^T . ST[n,m]  per b   (node -> feature)
PE transposes convert feature-layout layer outputs back to node layout.

Matmuls run in float32r (TF32-like, 1 cyc/row at free>=256) by default;
set CCRNN_FP32=1 for full-fp32 matmuls.
"""

import os
import numpy as np

B, T, N, H, K, GL, NPRED, NDIM = 16, 12, 512, 64, 3, 3, 12, 64
IN_DIM = OUT_DIM = 1
M = K + 1
NC_CORES = 8
BC = B // NC_CORES          # batch per core = 2
FREE = BC * N               # 1024
HF = 512                    # free half
NCH = N // 128              # 4 node chunks

_STATE = {}


# ----------------------------------------------------------------- host math
def _leaky_relu(x, alpha=0.01):
    return np.where(x >= 0, x, alpha * x).astype(np.float32)


def _softmax_rows(x):
    m = x.max(axis=-1, keepdims=True)
    e = np.exp((x - m).astype(np.float32))
    return (e / e.sum(axis=-1, keepdims=True)).astype(np.float32)


def _make_supports_host(p):
    nv1 = np.asarray(p["nodevec1"], np.float32)
    nv2 = np.asarray(p["nodevec2"], np.float32)
    w1 = np.asarray(p["w1"], np.float32)
    w2 = np.asarray(p["w2"], np.float32)
    b1 = np.asarray(p["b1"], np.float32)
    b2 = np.asarray(p["b2"], np.float32)
    g0 = _leaky_relu(nv1 @ nv2)
    nv1 = nv1 @ w1 + b1
    nv2 = ((nv2.T @ w1) + b1).T
    g1 = _leaky_relu(nv1 @ nv2)
    nv1 = nv1 @ w2 + b2
    nv2 = ((nv2.T @ w2) + b2).T
    g2 = _leaky_relu(nv1 @ nv2)
    return [_softmax_rows(g) for g in (g0, g1, g2)]


def _pack_cell(p, fdim, out_perm):
    """Pack one EvolutionCell's params into tile layouts.

    Cat feature order on chip is [state(0:64), x(64)] (reference is [x, state]);
    sigma maps our cat row -> reference feature index. out_perm permutes the
    cell's output features (used to put u at rows 0:64, r at 64:128 for 'ru').
    Returns wk [3,128,4*fdim], bias [fdim,3], attw [fdim,512], attb [1,1].
    """
    d0 = 1 + H
    sigma = np.array([1 + j for j in range(H)] + [0], np.int64)  # ours -> ref
    perm = np.arange(fdim) if out_perm is None else out_perm
    wk = np.zeros((GL, 128, M * fdim), np.float32)
    bias = np.zeros((fdim, GL), np.float32)
    for l in range(GL):
        W = np.asarray(p["W"][l], np.float32)          # [(d*M), fdim] rows (di, k)
        d = d0 if l == 0 else fdim
        W = W.reshape(d, M, fdim)
        rowmap = sigma if l == 0 else perm
        for k in range(M):
            # ours row j, ours col f  =  W[rowmap[j], k, perm[f]]
            wk[l, :d, k * fdim:(k + 1) * fdim] = W[rowmap][:, k][:, perm]
        bias[:, l] = np.asarray(p["b"][l], np.float32)[perm]
    attw = np.asarray(p["attW"], np.float32).reshape(N, fdim)[:, perm].T  # [fdim, N]
    attb = np.asarray(p["attb"], np.float32).reshape(1, 1)
    return wk, np.ascontiguousarray(bias), np.ascontiguousarray(attw), attb


def _prep_host(inputs, params):
    sups = _make_supports_host(params)
    uniq, supidx = [], []
    for s in sups:
        for i, u in enumerate(uniq):
            if np.array_equal(s, u):
                supidx.append(i)
                break
        else:
            supidx.append(len(uniq))
            uniq.append(s)
    # support-power tiles: sup_h[u][j][p, kc*512 + m] = (S_u^(j+1))[m, kc*128 + p]
    sup_h = np.zeros((len(uniq), 3, 128, NCH * N), np.float32)
    for u, s in enumerate(uniq):
        pw = s
        for j in range(3):
            st = pw.T  # [n, m]
            for kc in range(NCH):
                sup_h[u, j, :, kc * N:(kc + 1) * N] = st[kc * 128:(kc + 1) * 128]
            pw = (pw @ s).astype(np.float32)

    swap = np.concatenate([np.arange(H, 2 * H), np.arange(H)])  # u first, r second
    cells = {
        "er": _pack_cell(params["enc"]["ru"], 2 * H, swap),
        "ec": _pack_cell(params["enc"]["cand"], H, None),
        "dr": _pack_cell(params["dec"]["cell"]["ru"], 2 * H, swap),
        "dc": _pack_cell(params["dec"]["cell"]["cand"], H, None),
    }
    outw = np.asarray(params["dec"]["outW"], np.float32).reshape(H, 1)
    outb = np.asarray(params["dec"]["outb"], np.float32).reshape(1, 1)
    return sup_h, supidx, cells, outw, outb


# ----------------------------------------------------------------- bass build
def _build(n_uniq, supidx):
    import concourse.bacc as bacc
    import concourse.tile as tile
    import concourse.mybir as mybir

    fp32 = os.environ.get("CCRNN_FP32", "0") == "1"
    DT = mybir.dt.float32 if fp32 else mybir.dt.float32r
    F32 = mybir.dt.float32
    AF = mybir.ActivationFunctionType
    ALU = mybir.AluOpType
    AX = mybir.AxisListType

    nc = bacc.Bacc("TRN2", target_bir_lowering=False, debug=False,
                   num_devices=NC_CORES)

    # ---- dram
    d_sup = nc.dram_tensor("sup", (n_uniq, 3, 128, NCH * N), F32, kind="ExternalInput")
    d_xin = nc.dram_tensor("xin", (T, BC * N), F32, kind="ExternalInput")
    d_xn = nc.dram_tensor("xn", (NCH, 128, BC * T), F32, kind="ExternalInput")
    d_id = nc.dram_tensor("ident", (128, 128), F32, kind="ExternalInput")
    d_ones = nc.dram_tensor("ones", (128, 128), F32, kind="ExternalInput")
    d_zero = nc.dram_tensor("zero", (128, 1024), F32, kind="ExternalInput")
    d_wk, d_bi, d_aw, d_ab = {}, {}, {}, {}
    for cname, fdim in (("er", 2 * H), ("ec", H), ("dr", 2 * H), ("dc", H)):
        d_wk[cname] = nc.dram_tensor(f"wk_{cname}", (GL, 128, M * fdim), F32,
                                     kind="ExternalInput")
        d_bi[cname] = nc.dram_tensor(f"bi_{cname}", (fdim, GL), F32,
                                     kind="ExternalInput")
        d_aw[cname] = nc.dram_tensor(f"aw_{cname}", (fdim, N), F32,
                                     kind="ExternalInput")
        d_ab[cname] = nc.dram_tensor(f"ab_{cname}", (1, 1), F32,
                                     kind="ExternalInput")
    d_outw = nc.dram_tensor("outw", (H, 1), F32, kind="ExternalInput")
    d_outb = nc.dram_tensor("outb", (1, 1), F32, kind="ExternalInput")
    d_out = nc.dram_tensor("out", (NPRED, BC * N), F32, kind="ExternalOutput")

    with tile.TileContext(nc) as tc:
        with (
            tc.tile_pool(name="pers", bufs=1) as pp,
            tc.tile_pool(name="work", bufs=2) as wp,
            tc.tile_pool(name="psA", bufs=4, space="PSUM") as psA,
            tc.tile_pool(name="psT", bufs=2, space="PSUM") as psT,
            tc.tile_pool(name="psS", bufs=2, space="PSUM") as psS,
        ):
            # ---- persistent tiles
            SUP = []
            for u in range(n_uniq):
                row = []
                for j in range(3):
                    t = pp.tile([128, NCH * N], DT, tag=f"sup{u}_{j}",
                                name=f"sup{u}_{j}")
                    nc.gpsimd.dma_start(t[:], d_sup[u, j])
                    row.append(t)
                SUP.append(row)
            XN = []
            for c in range(NCH):
                t = pp.tile([128, BC * T], DT, tag=f"xn{c}")
                nc.gpsimd.dma_start(t[:], d_xn[c])
                XN.append(t)
            IDr = pp.tile([128, 128], DT, tag="idr")
            nc.gpsimd.dma_start(IDr[:], d_id[:])
            WK, BI, AW, ABt = {}, {}, {}, {}
            for cname, fdim in (("er", 2 * H), ("ec", H), ("dr", 2 * H), ("dc", H)):
                WK[cname] = []
                for l in range(GL):
                    t = pp.tile([128, M * fdim], DT, tag=f"wk{cname}{l}", name=f"wk{cname}{l}")
                    nc.gpsimd.dma_start(t[:], d_wk[cname][l])
                    WK[cname].append(t)
                BI[cname] = pp.tile([fdim, GL], F32, tag=f"bi{cname}", name=f"bi{cname}")
                nc.sync.dma_start(BI[cname][:], d_bi[cname][:])
                AW[cname] = pp.tile([fdim, N], F32, tag=f"aw{cname}", name=f"aw{cname}")
                nc.sync.dma_start(AW[cname][:], d_aw[cname][:])
                ABt[cname] = pp.tile([1, 1], F32, tag=f"ab{cname}", name=f"ab{cname}")
                nc.sync.dma_start(ABt[cname][:], d_ab[cname][:])
            OUTW = pp.tile([H, 1], DT, tag="outw")
            nc.gpsimd.dma_start(OUTW[:], d_outw[:])
            OUTB = pp.tile([1, 1], F32, tag="outb")
            nc.sync.dma_start(OUTB[:], d_outb[:])
            ONES = pp.tile([128, 1], F32, tag="ones")
            nc.sync.dma_start(ONES[:], d_ones[:, 0:1])
            ONESR = pp.tile([1, 128], F32, tag="onesr")
            nc.sync.dma_start(ONESR[:], d_ones[0:1, :])

            # activations / state (persistent, zero-init)
            def zt(name, p, f, dt=DT):
                t = pp.tile([p, f], dt, tag=name, name=name)
                if dt == F32:
                    nc.vector.memset(t[:], 0.0)
                else:
                    nc.gpsimd.dma_start(t[:], d_zero[0:p, 0:f])
                return t

            CATF = zt("catf", 1 + H, FREE)       # [state(0:64); x(64)] feature
            CATC = zt("catc", 1 + H, FREE)       # [r*state(0:64); x(64)]
            CATN_R = [zt(f"catnr{c}", 128, 256) for c in range(NCH)]
            CATN_C = [zt(f"catnc{c}", 128, 256) for c in range(NCH)]
            NCr = [zt(f"ncr{c}", 128, 256) for c in range(NCH)]  # ru layer-out node
            NDr = [zt(f"ndr{c}", 128, 256) for c in range(NCH)]
            NCc = [zt(f"ncc{c}", 128, 256) for c in range(NCH)]  # cand layer-out node
            NDc = [zt(f"ndc{c}", 128, 256) for c in range(NCH)]
            Y1 = zt("y1", 128, FREE)
            Y2 = zt("y2", 128, FREE)
            Y3 = zt("y3", 128, FREE)
            O0 = zt("o0", 128, FREE)
            O1 = zt("o1", 128, FREE)
            O2 = zt("o2", 128, FREE)
            OO = [O0, O1, O2]
            EVR = zt("evr", 128, FREE)
            EVC = zt("evc", H, FREE)
            RU = zt("ru", 128, FREE)
            CC = zt("cc", H, FREE)
            T1 = zt("t1", H, FREE)
            RT = zt("rt", H, FREE)
            DOUT = zt("dout", 1, FREE)
            SC = zt("sc", 128, 8, F32)
            WSC = zt("wsc", 1, 8, F32)
            WB = zt("wb", 128, 8, F32)
            SM1 = zt("sm1", 1, 8, F32)   # scratch rows: scores/exp
            SM2 = zt("sm2", 1, 8, F32)   # max / sum / recip packed

            def cheb(sups, x0n, d, y0f):
                """Y1=(S x0)^T, Y2=2(S^2 x0)^T - y0, Y3=4(S^3 x0)^T - 3 Y1."""
                for b in range(BC):
                    bs = slice(b * HF, (b + 1) * HF)
                    ps1 = psA.tile([128, 512], F32, tag="psA")
                    ps2 = psA.tile([128, 512], F32, tag="psA")
                    ps3 = psA.tile([128, 512], F32, tag="psA")
                    for kc in range(NCH):
                        lhs = x0n[kc][:, b * d:(b + 1) * d]
                        ks = slice(kc * N, (kc + 1) * N)
                        nc.tensor.matmul(ps1[0:d, :], lhs, sups[0][:, ks],
                                         start=(kc == 0), stop=(kc == NCH - 1))
                        nc.tensor.matmul(ps2[0:d, :], lhs, sups[1][:, ks],
                                         start=(kc == 0), stop=(kc == NCH - 1))
                        nc.tensor.matmul(ps3[0:d, :], lhs, sups[2][:, ks],
                                         start=(kc == 0), stop=(kc == NCH - 1))
                    nc.scalar.copy(Y1[0:d, bs], ps1[0:d, :])
                    nc.vector.scalar_tensor_tensor(
                        Y2[0:d, bs], ps2[0:d, :], 2.0, y0f[0:d, bs],
                        op0=ALU.mult, op1=ALU.subtract)
                    nc.vector.scalar_tensor_tensor(
                        Y3[0:d, bs], ps3[0:d, :], 4.0, Y1[0:d, bs],
                        op0=ALU.mult, op1=ALU.subtract)
                    nc.vector.scalar_tensor_tensor(
                        Y3[0:d, bs], Y1[0:d, bs], -2.0, Y3[0:d, bs],
                        op0=ALU.mult, op1=ALU.add)

            def evo(cname, fdim, catf, catn, ev_out):
                wk, bi, aw, ab = WK[cname], BI[cname], AW[cname], ABt[cname]
                for l in range(GL):
                    d = 1 + H if l == 0 else fdim
                    sups = SUP[supidx[l]]
                    if l == 0:
                        x0n, y0f = catn, catf
                    elif l == 1:
                        x0n = NCr if fdim == 2 * H else NCc
                        y0f = OO[0]
                    else:
                        x0n = NDr if fdim == 2 * H else NDc
                        y0f = OO[1]
                    cheb(sups, x0n, d, y0f)
                    ys = [y0f, Y1, Y2, Y3]
                    for h in range(2):
                        hs = slice(h * HF, (h + 1) * HF)
                        ps = psA.tile([128, 512], F32, tag="psA")
                        for k in range(M):
                            nc.tensor.matmul(
                                ps[0:fdim, :],
                                wk[l][0:d, k * fdim:(k + 1) * fdim],
                                ys[k][0:d, hs],
                                start=(k == 0), stop=(k == M - 1))
                        nc.scalar.activation(OO[l][0:fdim, hs], ps[0:fdim, :],
                                             AF.Identity, bias=bi[0:fdim, l:l + 1])
                    if l < 2:
                        dstn = (NCr if fdim == 2 * H else NCc) if l == 0 else \
                               (NDr if fdim == 2 * H else NDc)
                        for b in range(BC):
                            for cc in range(NCH):
                                pt = psT.tile([128, 128], DT, tag="psT")
                                nc.tensor.transpose(
                                    pt[:, 0:fdim],
                                    OO[l][0:fdim, b * HF + cc * 128: b * HF + cc * 128 + 128],
                                    IDr[0:fdim, 0:fdim])
                                nc.vector.tensor_copy(
                                    dstn[cc][:, b * fdim:(b + 1) * fdim],
                                    pt[:, 0:fdim])
                # attention over the three layer outputs
                for g in range(GL):
                    for b in range(BC):
                        prd = wp.tile([128, 512], F32, tag="prd")
                        bs = slice(b * HF, (b + 1) * HF)
                        col = b * GL + g
                        nc.vector.tensor_tensor_reduce(
                            prd[0:fdim, :], OO[g][0:fdim, bs], aw[0:fdim, :],
                            1.0, 0.0, op0=ALU.mult, op1=ALU.add,
                            accum_out=SC[0:fdim, col:col + 1])
                pss = psS.tile([1, 8], F32, tag="psS")
                nc.tensor.matmul(pss[0:1, 0:2 * GL], ONES[0:fdim, 0:1],
                                 SC[0:fdim, 0:2 * GL], start=True, stop=True)
                nc.scalar.activation(SM1[0:1, 0:2 * GL], pss[0:1, 0:2 * GL],
                                     AF.Identity, bias=ab[0:1, 0:1])
                s3 = SM1[0:1, 0:2 * GL].rearrange("p (b g) -> p b g", g=GL)
                nc.vector.reduce_max(SM2[0:1, 0:BC], s3, axis=AX.X)
                mxb = SM2[0:1, 0:BC].unsqueeze(2).broadcast_to([1, BC, GL])
                nc.vector.tensor_sub(s3, s3, mxb)
                nc.scalar.activation(SM1[0:1, 0:2 * GL], SM1[0:1, 0:2 * GL], AF.Exp)
                nc.vector.reduce_sum(SM2[0:1, 4:4 + BC], s3, axis=AX.X)
                nc.vector.reciprocal(SM2[0:1, 6:6 + BC], SM2[0:1, 4:4 + BC])
                rcb = SM2[0:1, 6:6 + BC].unsqueeze(2).broadcast_to([1, BC, GL])
                nc.vector.tensor_mul(WSC[0:1, 0:2 * GL].rearrange(
                    "p (b g) -> p b g", g=GL), s3, rcb)
                psb = psS.tile([128, 8], F32, tag="psS")
                nc.tensor.matmul(psb[:, 0:2 * GL], ONESR[0:1, :],
                                 WSC[0:1, 0:2 * GL], start=True, stop=True)
                nc.vector.tensor_copy(WB[:, 0:2 * GL], psb[:, 0:2 * GL])
                for b in range(BC):
                    bs = slice(b * HF, (b + 1) * HF)
                    nc.vector.tensor_scalar_mul(
                        ev_out[0:fdim, bs], OO[0][0:fdim, bs],
                        WB[0:fdim, b * GL:b * GL + 1])
                    for g in (1, 2):
                        nc.vector.scalar_tensor_tensor(
                            ev_out[0:fdim, bs], OO[g][0:fdim, bs],
                            WB[0:fdim, b * GL + g:b * GL + g + 1],
                            ev_out[0:fdim, bs], op0=ALU.mult, op1=ALU.add)

            def state_transposes(srcF, dstn):
                for b in range(BC):
                    for cc in range(NCH):
                        pt = psT.tile([128, 128], DT, tag="psT")
                        nc.tensor.transpose(
                            pt[:, 0:H],
                            srcF[0:H, b * HF + cc * 128: b * HF + cc * 128 + 128],
                            IDr[0:H, 0:H])
                        nc.vector.tensor_copy(
                            dstn[cc][:, b * (1 + H): b * (1 + H) + H], pt[:, 0:H])

            def dcgru(cr, cc_, is_dec, t):
                # x row into cat tiles (feature layout)
                if not is_dec:
                    nc.gpsimd.dma_start(CATF[H:H + 1, :], d_xin[t:t + 1, :])
                    nc.gpsimd.dma_start(CATC[H:H + 1, :], d_xin[t:t + 1, :])
                else:
                    nc.sync.dma_start(CATF[H:H + 1, :], DOUT[0:1, :])
                    nc.sync.dma_start(CATC[H:H + 1, :], DOUT[0:1, :])
                # node-layout cat_ru: state^T cols + x col
                state_transposes(CATF, CATN_R)
                for b in range(BC):
                    for ccn in range(NCH):
                        xcol = slice(b * (1 + H) + H, b * (1 + H) + H + 1)
                        if not is_dec:
                            src = XN[ccn][:, b * T + t: b * T + t + 1]
                            nc.vector.tensor_copy(CATN_R[ccn][:, xcol], src)
                            nc.vector.tensor_copy(CATN_C[ccn][:, xcol], src)
                        else:
                            po = psT.tile([128, 128], F32, tag="psT")
                            nc.tensor.matmul(
                                po[:, 0:1],
                                DOUT[0:1, b * HF + ccn * 128: b * HF + ccn * 128 + 128].bitcast(F32),
                                ONES[0:1, 0:1], start=True, stop=True)
                            nc.vector.tensor_copy(CATN_R[ccn][:, xcol], po[:, 0:1])
                            nc.vector.tensor_copy(CATN_C[ccn][:, xcol], po[:, 0:1])
                evo(cr, 2 * H, CATF, CATN_R, EVR)
                nc.scalar.activation(RU[:], EVR[:], AF.Sigmoid)
                # r (rows 64:128 after swap): partition-shift via DMA, then mul
                nc.sync.dma_start(RT[0:H, :], RU[H:2 * H, :])
                nc.vector.tensor_mul(CATC[0:H, :], RT[0:H, :], CATF[0:H, :])
                state_transposes(CATC, CATN_C)
                evo(cc_, H, CATC, CATN_C, EVC)
                nc.scalar.activation(CC[0:H, :], EVC[0:H, :], AF.Tanh)
                # state' = c + u*(state - c);  u = RU[0:64]
                nc.vector.tensor_sub(T1[0:H, :], CATF[0:H, :], CC[0:H, :])
                nc.vector.tensor_mul(T1[0:H, :], T1[0:H, :], RU[0:H, :])
                nc.vector.tensor_add(CATF[0:H, :], T1[0:H, :], CC[0:H, :])

            trivial = os.environ.get("CCRNN_TRIVIAL", "0") == "1"
            for t in range(0 if trivial else T):
                dcgru("er", "ec", False, t)
            for i in range(0 if trivial else NPRED):
                dcgru("dr", "dc", True, i)
                for h in range(2):
                    hs = slice(h * HF, (h + 1) * HF)
                    ps = psA.tile([128, 512], F32, tag="psA")
                    nc.tensor.matmul(ps[0:1, :], OUTW[0:H, 0:1], CATF[0:H, hs],
                                     start=True, stop=True)
                    nc.scalar.activation(DOUT[0:1, hs], ps[0:1, :],
                                         AF.Identity, bias=OUTB[0:1, 0:1])
                import concourse.mybir as _mb
                nc.sync.dma_start(d_out[i:i + 1, :],
                                  DOUT[0:1, :].bitcast(_mb.dt.float32))
            if trivial:
                import concourse.mybir as _mb
                for i in range(NPRED):
                    nc.sync.dma_start(d_out[i:i + 1, :],
                                      DOUT[0:1, :].bitcast(_mb.dt.float32))

    nc.compile()
    return nc


# ----------------------------------------------------------------- entry
def kernel(inputs, params):
    from concourse.bass_utils import run_bass_kernel_spmd

    inputs = np.asarray(inputs, np.float32)
    sup_h, supidx, cells, outw, outb = _prep_host(inputs, params)

    key = (sup_h.shape[0], tuple(supidx), os.environ.get("CCRNN_FP32", "0"),
           os.environ.get("CCRNN_TRIVIAL", "0"))
    if _STATE.get("key") != key:
        _STATE["nc"] = _build(sup_h.shape[0], supidx)
        _STATE["key"] = key
    nc = _STATE["nc"]

    ident = np.eye(128, dtype=np.float32)
    base = {
        "sup": np.ascontiguousarray(sup_h),
        "ident": ident,
        "ones": np.ones((128, 128), np.float32),
        "zero": np.zeros((128, 1024), np.float32),
        "outw": np.ascontiguousarray(outw), "outb": np.ascontiguousarray(outb),
    }
    for cname in ("er", "ec", "dr", "dc"):
        wk, bi, aw, ab = cells[cname]
        base[f"wk_{cname}"] = np.ascontiguousarray(wk)
        base[f"bi_{cname}"] = bi
        base[f"aw_{cname}"] = aw
        base[f"ab_{cname}"] = ab

    x = inputs[..., 0]                                   # [B, T, N]
    in_maps = []
    for c in range(NC_CORES):
        xs = x[c * BC:(c + 1) * BC]                      # [BC, T, N]
        xin = np.ascontiguousarray(
            np.transpose(xs, (1, 0, 2)).reshape(T, BC * N))
        xn = np.zeros((NCH, 128, BC * T), np.float32)
        for ch in range(NCH):
            blk = xs[:, :, ch * 128:(ch + 1) * 128]      # [BC, T, 128]
            xn[ch] = np.transpose(blk, (2, 0, 1)).reshape(128, BC * T)
        in_maps.append({**base, "xin": xin, "xn": np.ascontiguousarray(xn)})

    res = run_bass_kernel_spmd(nc, in_maps, core_ids=list(range(NC_CORES)))
    out = np.zeros((B, NPRED, N, 1), np.float32)
    for c in range(NC_CORES):
        o = res.results[c]["out"].reshape(NPRED, BC, N)
        out[c * BC:(c + 1) * BC] = np.transpose(o, (1, 0, 2))[..., None]
    return out


# revision 15
# speedup vs baseline: 4.8675x; 4.8675x over previous
"""CCRNN (gnn_message_passing) Trainium2 Bass kernel.

Data-parallel over batch B=16 across 8 NeuronCores (2 per core). The whole
recurrence runs SBUF-resident. Two on-chip layouts are maintained:
  node layout    [n_chunk][128 nodes, (b, d) free]   -> graph-diffusion matmuls
  feature layout [d features (partitions), (b, n)]   -> feature matmuls / gates
Graph diffusion (Chebyshev) uses two matmul forms, both consuming node layout:
  form-1: out[m,(b,d)] = sum_n ST[n,m]^T . x[n,(b,d)]      (node -> node)
  form-2: out[d,m]     = sum_n x[n,d]^T . ST[n,m]  per b   (node -> feature)
PE transposes convert feature-layout layer outputs back to node layout.

Matmuls run in float32r (TF32-like, 1 cyc/row at free>=256) by default;
set CCRNN_FP32=1 for full-fp32 matmuls.
"""

import os
import numpy as np

B, T, N, H, K, GL, NPRED, NDIM = 16, 12, 512, 64, 3, 3, 12, 64
IN_DIM = OUT_DIM = 1
M = K + 1
NC_CORES = 8
BC = B // NC_CORES          # batch per core = 2
FREE = BC * N               # 1024
HF = 512                    # free half
NCH = N // 128              # 4 node chunks

_STATE = {}


# ----------------------------------------------------------------- host math
def _leaky_relu(x, alpha=0.01):
    return np.where(x >= 0, x, alpha * x).astype(np.float32)


def _softmax_rows(x):
    m = x.max(axis=-1, keepdims=True)
    e = np.exp((x - m).astype(np.float32))
    return (e / e.sum(axis=-1, keepdims=True)).astype(np.float32)


def _make_supports_host(p):
    nv1 = np.asarray(p["nodevec1"], np.float32)
    nv2 = np.asarray(p["nodevec2"], np.float32)
    w1 = np.asarray(p["w1"], np.float32)
    w2 = np.asarray(p["w2"], np.float32)
    b1 = np.asarray(p["b1"], np.float32)
    b2 = np.asarray(p["b2"], np.float32)
    g0 = _leaky_relu(nv1 @ nv2)
    nv1 = nv1 @ w1 + b1
    nv2 = ((nv2.T @ w1) + b1).T
    g1 = _leaky_relu(nv1 @ nv2)
    nv1 = nv1 @ w2 + b2
    nv2 = ((nv2.T @ w2) + b2).T
    g2 = _leaky_relu(nv1 @ nv2)
    return [_softmax_rows(g) for g in (g0, g1, g2)]


def _pack_cell(p, fdim, out_perm):
    """Pack one EvolutionCell's params into tile layouts.

    Cat feature order on chip is [state(0:64), x(64)] (reference is [x, state]);
    sigma maps our cat row -> reference feature index. out_perm permutes the
    cell's output features (used to put u at rows 0:64, r at 64:128 for 'ru').
    Returns wk [3,128,4*fdim], bias [fdim,3], attw [fdim,512], attb [1,1].
    """
    d0 = 1 + H
    sigma = np.array([1 + j for j in range(H)] + [0], np.int64)  # ours -> ref
    perm = np.arange(fdim) if out_perm is None else out_perm
    wk = np.zeros((GL, 128, M * fdim), np.float32)
    bias = np.zeros((fdim, GL), np.float32)
    for l in range(GL):
        W = np.asarray(p["W"][l], np.float32)          # [(d*M), fdim] rows (di, k)
        d = d0 if l == 0 else fdim
        W = W.reshape(d, M, fdim)
        rowmap = sigma if l == 0 else perm
        for k in range(M):
            # ours row j, ours col f  =  W[rowmap[j], k, perm[f]]
            wk[l, :d, k * fdim:(k + 1) * fdim] = W[rowmap][:, k][:, perm]
        bias[:, l] = np.asarray(p["b"][l], np.float32)[perm]
    attw = np.asarray(p["attW"], np.float32).reshape(N, fdim)[:, perm].T  # [fdim, N]
    attb = np.asarray(p["attb"], np.float32).reshape(1, 1)
    return wk, np.ascontiguousarray(bias), np.ascontiguousarray(attw), attb


def _prep_host(inputs, params):
    sups = _make_supports_host(params)
    uniq, supidx = [], []
    for s in sups:
        for i, u in enumerate(uniq):
            if np.array_equal(s, u):
                supidx.append(i)
                break
        else:
            supidx.append(len(uniq))
            uniq.append(s)
    # support-power tiles: sup_h[u][j][p, kc*512 + m] = (S_u^(j+1))[m, kc*128 + p]
    sup_h = np.zeros((len(uniq), 3, 128, NCH * N), np.float32)
    for u, s in enumerate(uniq):
        pw = s
        for j in range(3):
            st = pw.T  # [n, m]
            for kc in range(NCH):
                sup_h[u, j, :, kc * N:(kc + 1) * N] = st[kc * 128:(kc + 1) * 128]
            pw = (pw @ s).astype(np.float32)

    swap = np.concatenate([np.arange(H, 2 * H), np.arange(H)])  # u first, r second
    cells = {
        "er": _pack_cell(params["enc"]["ru"], 2 * H, swap),
        "ec": _pack_cell(params["enc"]["cand"], H, None),
        "dr": _pack_cell(params["dec"]["cell"]["ru"], 2 * H, swap),
        "dc": _pack_cell(params["dec"]["cell"]["cand"], H, None),
    }
    outw = np.asarray(params["dec"]["outW"], np.float32).reshape(H, 1)
    outb = np.asarray(params["dec"]["outb"], np.float32).reshape(1, 1)
    return sup_h, supidx, cells, outw, outb


# ----------------------------------------------------------------- bass build
def _build(n_uniq, supidx):
    import concourse.bacc as bacc
    import concourse.tile as tile
    import concourse.mybir as mybir

    fp32 = os.environ.get("CCRNN_FP32", "0") == "1"
    DT = mybir.dt.float32 if fp32 else mybir.dt.float32r
    F32 = mybir.dt.float32
    AF = mybir.ActivationFunctionType
    ALU = mybir.AluOpType
    AX = mybir.AxisListType

    nc = bacc.Bacc("TRN2", target_bir_lowering=False, debug=False,
                   num_devices=NC_CORES)

    # ---- dram
    d_sup = nc.dram_tensor("sup", (n_uniq, 3, 128, NCH * N), F32, kind="ExternalInput")
    d_xin = nc.dram_tensor("xin", (T, BC * N), F32, kind="ExternalInput")
    d_xn = nc.dram_tensor("xn", (NCH, 128, BC * T), F32, kind="ExternalInput")
    d_id = nc.dram_tensor("ident", (128, 128), F32, kind="ExternalInput")
    d_ones = nc.dram_tensor("ones", (128, 128), F32, kind="ExternalInput")
    d_zero = nc.dram_tensor("zero", (128, 1024), F32, kind="ExternalInput")
    d_wk, d_bi, d_aw, d_ab = {}, {}, {}, {}
    for cname, fdim in (("er", 2 * H), ("ec", H), ("dr", 2 * H), ("dc", H)):
        d_wk[cname] = nc.dram_tensor(f"wk_{cname}", (GL, 128, M * fdim), F32,
                                     kind="ExternalInput")
        d_bi[cname] = nc.dram_tensor(f"bi_{cname}", (fdim, GL), F32,
                                     kind="ExternalInput")
        d_aw[cname] = nc.dram_tensor(f"aw_{cname}", (fdim, N), F32,
                                     kind="ExternalInput")
        d_ab[cname] = nc.dram_tensor(f"ab_{cname}", (1, 1), F32,
                                     kind="ExternalInput")
    d_outw = nc.dram_tensor("outw", (H, 1), F32, kind="ExternalInput")
    d_outb = nc.dram_tensor("outb", (1, 1), F32, kind="ExternalInput")
    d_out = nc.dram_tensor("out", (NPRED, BC * N), F32, kind="ExternalOutput")

    with tile.TileContext(nc) as tc:
        with (
            tc.tile_pool(name="pers", bufs=1) as pp,
            tc.tile_pool(name="work", bufs=2) as wp,
            tc.tile_pool(name="psA", bufs=4, space="PSUM") as psA,
            tc.tile_pool(name="psT", bufs=2, space="PSUM") as psT,
            tc.tile_pool(name="psS", bufs=2, space="PSUM") as psS,
        ):
            # ---- persistent tiles
            SUP = []
            for u in range(n_uniq):
                row = []
                for j in range(3):
                    t = pp.tile([128, NCH * N], DT, tag=f"sup{u}_{j}",
                                name=f"sup{u}_{j}")
                    nc.gpsimd.dma_start(t[:], d_sup[u, j])
                    row.append(t)
                SUP.append(row)
            XN = []
            for c in range(NCH):
                t = pp.tile([128, BC * T], DT, tag=f"xn{c}")
                nc.gpsimd.dma_start(t[:], d_xn[c])
                XN.append(t)
            IDr = pp.tile([128, 128], DT, tag="idr")
            nc.gpsimd.dma_start(IDr[:], d_id[:])
            WK, BI, AW, ABt = {}, {}, {}, {}
            for cname, fdim in (("er", 2 * H), ("ec", H), ("dr", 2 * H), ("dc", H)):
                WK[cname] = []
                for l in range(GL):
                    t = pp.tile([128, M * fdim], DT, tag=f"wk{cname}{l}", name=f"wk{cname}{l}")
                    nc.gpsimd.dma_start(t[:], d_wk[cname][l])
                    WK[cname].append(t)
                BI[cname] = pp.tile([fdim, GL], F32, tag=f"bi{cname}", name=f"bi{cname}")
                nc.sync.dma_start(BI[cname][:], d_bi[cname][:])
                AW[cname] = pp.tile([fdim, N], F32, tag=f"aw{cname}", name=f"aw{cname}")
                nc.sync.dma_start(AW[cname][:], d_aw[cname][:])
                ABt[cname] = pp.tile([1, 1], F32, tag=f"ab{cname}", name=f"ab{cname}")
                nc.sync.dma_start(ABt[cname][:], d_ab[cname][:])
            OUTW = pp.tile([H, 1], DT, tag="outw")
            nc.gpsimd.dma_start(OUTW[:], d_outw[:])
            OUTB = pp.tile([1, 1], F32, tag="outb")
            nc.sync.dma_start(OUTB[:], d_outb[:])
            ONES = pp.tile([128, 1], F32, tag="ones")
            nc.sync.dma_start(ONES[:], d_ones[:, 0:1])
            ONESR = pp.tile([1, 128], F32, tag="onesr")
            nc.sync.dma_start(ONESR[:], d_ones[0:1, :])

            # activations / state (persistent, zero-init)
            def zt(name, p, f, dt=DT):
                t = pp.tile([p, f], dt, tag=name, name=name)
                if dt == F32:
                    nc.vector.memset(t[:], 0.0)
                else:
                    nc.gpsimd.dma_start(t[:], d_zero[0:p, 0:f])
                return t

            CATF = zt("catf", 1 + H, FREE)       # [state(0:64); x(64)] feature
            CATC = zt("catc", 1 + H, FREE)       # [r*state(0:64); x(64)]
            CATN_R = [zt(f"catnr{c}", 128, 256) for c in range(NCH)]
            CATN_C = [zt(f"catnc{c}", 128, 256) for c in range(NCH)]
            NCr = [zt(f"ncr{c}", 128, 256) for c in range(NCH)]  # ru layer-out node
            NDr = [zt(f"ndr{c}", 128, 256) for c in range(NCH)]
            NCc = [zt(f"ncc{c}", 128, 256) for c in range(NCH)]  # cand layer-out node
            NDc = [zt(f"ndc{c}", 128, 256) for c in range(NCH)]
            Y1 = zt("y1", 128, FREE)
            Y2 = zt("y2", 128, FREE)
            Y3 = zt("y3", 128, FREE)
            O0 = zt("o0", 128, FREE)
            O1 = zt("o1", 128, FREE)
            O2 = zt("o2", 128, FREE)
            OO = [O0, O1, O2]
            EVR = zt("evr", 128, FREE)
            EVC = zt("evc", H, FREE)
            RU = zt("ru", 128, FREE)
            CC = zt("cc", H, FREE)
            T1 = zt("t1", H, FREE)
            RT = zt("rt", H, FREE)
            DOUT = zt("dout", 1, FREE)
            SC = zt("sc", 128, 8, F32)
            WSC = zt("wsc", 1, 8, F32)
            WB = zt("wb", 128, 8, F32)
            SM1 = zt("sm1", 1, 8, F32)   # scratch rows: scores/exp
            SM2 = zt("sm2", 1, 8, F32)   # max / sum / recip packed

            def cheb(sups, x0n, d, y0f):
                """Y1=(S x0)^T, Y2=2(S^2 x0)^T - y0, Y3=4(S^3 x0)^T - 3 Y1."""
                for b in range(BC):
                    bs = slice(b * HF, (b + 1) * HF)
                    for j in range(3):
                        ps = psA.tile([128, 512], F32, tag="psA", name="ps")
                        for kc in range(NCH):
                            nc.tensor.matmul(
                                ps[0:d, :], x0n[kc][:, b * d:(b + 1) * d],
                                sups[j][:, kc * N:(kc + 1) * N],
                                start=(kc == 0), stop=(kc == NCH - 1))
                        if j == 0:
                            nc.vector.tensor_copy(Y1[0:d, bs], ps[0:d, :])
                        elif j == 1:
                            nc.vector.scalar_tensor_tensor(
                                Y2[0:d, bs], ps[0:d, :], 2.0, y0f[0:d, bs],
                                op0=ALU.mult, op1=ALU.subtract)
                        else:
                            nc.vector.scalar_tensor_tensor(
                                Y3[0:d, bs], ps[0:d, :], 4.0, Y1[0:d, bs],
                                op0=ALU.mult, op1=ALU.subtract)
                            nc.vector.scalar_tensor_tensor(
                                Y3[0:d, bs], Y1[0:d, bs], -2.0, Y3[0:d, bs],
                                op0=ALU.mult, op1=ALU.add)

            def evo(cname, fdim, catf, catn, ev_out):
                wk, bi, aw, ab = WK[cname], BI[cname], AW[cname], ABt[cname]
                for l in range(GL):
                    d = 1 + H if l == 0 else fdim
                    sups = SUP[supidx[l]]
                    if l == 0:
                        x0n, y0f = catn, catf
                    elif l == 1:
                        x0n = NCr if fdim == 2 * H else NCc
                        y0f = OO[0]
                    else:
                        x0n = NDr if fdim == 2 * H else NDc
                        y0f = OO[1]
                    cheb(sups, x0n, d, y0f)
                    ys = [y0f, Y1, Y2, Y3]
                    for h in range(2):
                        hs = slice(h * HF, (h + 1) * HF)
                        ps = psA.tile([128, 512], F32, tag="psA")
                        for k in range(M):
                            nc.tensor.matmul(
                                ps[0:fdim, :],
                                wk[l][0:d, k * fdim:(k + 1) * fdim],
                                ys[k][0:d, hs],
                                start=(k == 0), stop=(k == M - 1))
                        nc.scalar.activation(OO[l][0:fdim, hs], ps[0:fdim, :],
                                             AF.Identity, bias=bi[0:fdim, l:l + 1])
                    if l < 2:
                        dstn = (NCr if fdim == 2 * H else NCc) if l == 0 else \
                               (NDr if fdim == 2 * H else NDc)
                        for b in range(BC):
                            for cc in range(NCH):
                                pt = psT.tile([128, 128], DT, tag="psT")
                                nc.tensor.transpose(
                                    pt[:, 0:fdim],
                                    OO[l][0:fdim, b * HF + cc * 128: b * HF + cc * 128 + 128],
                                    IDr[0:fdim, 0:fdim])
                                nc.vector.tensor_copy(
                                    dstn[cc][:, b * fdim:(b + 1) * fdim],
                                    pt[:, 0:fdim])
                # attention over the three layer outputs
                for g in range(GL):
                    for b in range(BC):
                        prd = wp.tile([128, 512], F32, tag="prd")
                        bs = slice(b * HF, (b + 1) * HF)
                        col = b * GL + g
                        nc.vector.tensor_mul(prd[0:fdim, :],
                                             OO[g][0:fdim, bs].bitcast(F32),
                                             aw[0:fdim, :])
                        nc.vector.reduce_sum(SC[0:fdim, col:col + 1],
                                             prd[0:fdim, :], axis=AX.X)
                pss = psS.tile([1, 8], F32, tag="psS")
                nc.tensor.matmul(pss[0:1, 0:2 * GL], ONES[0:fdim, 0:1],
                                 SC[0:fdim, 0:2 * GL], start=True, stop=True)
                nc.scalar.activation(SM1[0:1, 0:2 * GL], pss[0:1, 0:2 * GL],
                                     AF.Identity, bias=ab[0:1, 0:1])
                s3 = SM1[0:1, 0:2 * GL].rearrange("p (b g) -> p b g", g=GL)
                nc.vector.reduce_max(SM2[0:1, 0:BC], s3, axis=AX.X)
                mxb = SM2[0:1, 0:BC].unsqueeze(2).broadcast_to([1, BC, GL])
                nc.vector.tensor_sub(s3, s3, mxb)
                nc.scalar.activation(SM1[0:1, 0:2 * GL], SM1[0:1, 0:2 * GL], AF.Exp)
                nc.vector.reduce_sum(SM2[0:1, 4:4 + BC], s3, axis=AX.X)
                nc.vector.reciprocal(SM2[0:1, 6:6 + BC], SM2[0:1, 4:4 + BC])
                rcb = SM2[0:1, 6:6 + BC].unsqueeze(2).broadcast_to([1, BC, GL])
                nc.vector.tensor_mul(WSC[0:1, 0:2 * GL].rearrange(
                    "p (b g) -> p b g", g=GL), s3, rcb)
                psb = psS.tile([128, 8], F32, tag="psS")
                nc.tensor.matmul(psb[:, 0:2 * GL], ONESR[0:1, :],
                                 WSC[0:1, 0:2 * GL], start=True, stop=True)
                nc.vector.tensor_copy(WB[:, 0:2 * GL], psb[:, 0:2 * GL])
                for b in range(BC):
                    bs = slice(b * HF, (b + 1) * HF)
                    nc.vector.tensor_scalar_mul(
                        ev_out[0:fdim, bs], OO[0][0:fdim, bs],
                        WB[0:fdim, b * GL:b * GL + 1])
                    for g in (1, 2):
                        nc.vector.scalar_tensor_tensor(
                            ev_out[0:fdim, bs], OO[g][0:fdim, bs],
                            WB[0:fdim, b * GL + g:b * GL + g + 1],
                            ev_out[0:fdim, bs], op0=ALU.mult, op1=ALU.add)

            def state_transposes(srcF, dstn):
                for b in range(BC):
                    for cc in range(NCH):
                        pt = psT.tile([128, 128], DT, tag="psT")
                        nc.tensor.transpose(
                            pt[:, 0:H],
                            srcF[0:H, b * HF + cc * 128: b * HF + cc * 128 + 128],
                            IDr[0:H, 0:H])
                        nc.vector.tensor_copy(
                            dstn[cc][:, b * (1 + H): b * (1 + H) + H], pt[:, 0:H])

            def dcgru(cr, cc_, is_dec, t):
                # x row into cat tiles (feature layout)
                if not is_dec:
                    nc.gpsimd.dma_start(CATF[H:H + 1, :], d_xin[t:t + 1, :])
                    nc.gpsimd.dma_start(CATC[H:H + 1, :], d_xin[t:t + 1, :])
                else:
                    nc.sync.dma_start(CATF[H:H + 1, :], DOUT[0:1, :])
                    nc.sync.dma_start(CATC[H:H + 1, :], DOUT[0:1, :])
                # node-layout cat_ru: state^T cols + x col
                state_transposes(CATF, CATN_R)
                for b in range(BC):
                    for ccn in range(NCH):
                        xcol = slice(b * (1 + H) + H, b * (1 + H) + H + 1)
                        if not is_dec:
                            src = XN[ccn][:, b * T + t: b * T + t + 1]
                            nc.vector.tensor_copy(CATN_R[ccn][:, xcol], src)
                            nc.vector.tensor_copy(CATN_C[ccn][:, xcol], src)
                        else:
                            po = psT.tile([128, 128], F32, tag="psT")
                            nc.tensor.matmul(
                                po[:, 0:1],
                                DOUT[0:1, b * HF + ccn * 128: b * HF + ccn * 128 + 128].bitcast(F32),
                                ONES[0:1, 0:1], start=True, stop=True)
                            nc.vector.tensor_copy(CATN_R[ccn][:, xcol], po[:, 0:1])
                            nc.vector.tensor_copy(CATN_C[ccn][:, xcol], po[:, 0:1])
                evo(cr, 2 * H, CATF, CATN_R, EVR)
                nc.scalar.activation(RU[:], EVR[:], AF.Sigmoid)
                # r (rows 64:128 after swap): partition-shift via DMA, then mul
                nc.sync.dma_start(RT[0:H, :], RU[H:2 * H, :])
                nc.vector.tensor_mul(CATC[0:H, :], RT[0:H, :], CATF[0:H, :])
                state_transposes(CATC, CATN_C)
                evo(cc_, H, CATC, CATN_C, EVC)
                nc.scalar.activation(CC[0:H, :], EVC[0:H, :], AF.Tanh)
                # state' = c + u*(state - c);  u = RU[0:64]
                nc.vector.tensor_sub(T1[0:H, :], CATF[0:H, :], CC[0:H, :])
                nc.vector.tensor_mul(T1[0:H, :], T1[0:H, :], RU[0:H, :])
                nc.vector.tensor_add(CATF[0:H, :], T1[0:H, :], CC[0:H, :])

            trivial = os.environ.get("CCRNN_TRIVIAL", "0") == "1"
            for t in range(0 if trivial else T):
                dcgru("er", "ec", False, t)
            for i in range(0 if trivial else NPRED):
                dcgru("dr", "dc", True, i)
                for h in range(2):
                    hs = slice(h * HF, (h + 1) * HF)
                    ps = psA.tile([128, 512], F32, tag="psA")
                    nc.tensor.matmul(ps[0:1, :], OUTW[0:H, 0:1], CATF[0:H, hs],
                                     start=True, stop=True)
                    nc.scalar.activation(DOUT[0:1, hs], ps[0:1, :],
                                         AF.Identity, bias=OUTB[0:1, 0:1])
                import concourse.mybir as _mb
                nc.sync.dma_start(d_out[i:i + 1, :],
                                  DOUT[0:1, :].bitcast(_mb.dt.float32))
            if trivial:
                import concourse.mybir as _mb
                for i in range(NPRED):
                    nc.sync.dma_start(d_out[i:i + 1, :],
                                      DOUT[0:1, :].bitcast(_mb.dt.float32))

    nc.compile()
    return nc


# ----------------------------------------------------------------- runner
def _make_runner(nc):
    """Persistent jitted SPMD executor (NEFF ships to the terminal once)."""
    import jax
    import numpy as np
    from jax.experimental.shard_map import shard_map
    from jax.sharding import Mesh, PartitionSpec
    import concourse.mybir as mybir
    from concourse import bass2jax
    bass2jax.install_neuronx_cc_hook()

    partition_name = (nc.partition_id_tensor.name
                      if nc.partition_id_tensor else None)
    in_names, out_names, out_avals, zero_outs = [], [], [], []
    for alloc in nc.m.functions[0].allocations:
        if not isinstance(alloc, mybir.MemoryLocationSet):
            continue
        name = alloc.memorylocations[0].name
        if alloc.kind == "ExternalInput":
            if name != partition_name:
                in_names.append(name)
        elif alloc.kind == "ExternalOutput":
            shape = tuple(alloc.tensor_shape)
            dtype = mybir.dt.np(alloc.dtype)
            out_names.append(name)
            out_avals.append(jax.core.ShapedArray(shape, dtype))
            zero_outs.append(np.zeros(shape, dtype))
    n_params = len(in_names)
    n_outs = len(out_avals)
    all_names = in_names + out_names + ([partition_name] if partition_name else [])

    def _body(*args):
        operands = list(args)
        if partition_name is not None:
            operands.append(bass2jax.partition_id_tensor())
        return tuple(bass2jax._bass_exec_p.bind(
            *operands, out_avals=tuple(out_avals), in_names=tuple(all_names),
            out_names=tuple(out_names), lowering_input_output_aliases=(),
            sim_require_finite=True, sim_require_nnan=True, nc=nc))

    devices = jax.devices()[:NC_CORES]
    mesh = Mesh(np.asarray(devices), ("core",))
    in_specs = (PartitionSpec("core"),) * (n_params + n_outs)
    out_specs = (PartitionSpec("core"),) * n_outs
    donate = tuple(range(n_params, n_params + n_outs))
    sharded = jax.jit(
        shard_map(_body, mesh=mesh, in_specs=in_specs, out_specs=out_specs,
                  check_rep=False),
        donate_argnums=donate, keep_unused=True)

    def run(in_maps):
        concat_in = [np.concatenate([np.asarray(m[nm]) for m in in_maps], axis=0)
                     for nm in in_names]
        concat_zeros = [np.zeros((NC_CORES * z.shape[0], *z.shape[1:]), z.dtype)
                        for z in zero_outs]
        outs = sharded(*concat_in, *concat_zeros)
        return [{nm: np.asarray(outs[i]).reshape(NC_CORES, *out_avals[i].shape)[c]
                 for i, nm in enumerate(out_names)}
                for c in range(NC_CORES)]

    return run


# ----------------------------------------------------------------- entry
def kernel(inputs, params):

    inputs = np.asarray(inputs, np.float32)
    sup_h, supidx, cells, outw, outb = _prep_host(inputs, params)

    key = (sup_h.shape[0], tuple(supidx), os.environ.get("CCRNN_FP32", "0"),
           os.environ.get("CCRNN_TRIVIAL", "0"))
    if _STATE.get("key") != key:
        nc = _build(sup_h.shape[0], supidx)
        if os.environ.get("CCRNN_OLDRUN", "0") == "1":
            from concourse.bass_utils import run_bass_kernel_spmd
            _STATE["run"] = lambda maps: run_bass_kernel_spmd(
                nc, maps, core_ids=list(range(NC_CORES))).results
        else:
            _STATE["run"] = _make_runner(nc)
        _STATE["key"] = key
    run = _STATE["run"]

    ident = np.eye(128, dtype=np.float32)
    base = {
        "sup": np.ascontiguousarray(sup_h),
        "ident": ident,
        "ones": np.ones((128, 128), np.float32),
        "zero": np.zeros((128, 1024), np.float32),
        "outw": np.ascontiguousarray(outw), "outb": np.ascontiguousarray(outb),
    }
    for cname in ("er", "ec", "dr", "dc"):
        wk, bi, aw, ab = cells[cname]
        base[f"wk_{cname}"] = np.ascontiguousarray(wk)
        base[f"bi_{cname}"] = bi
        base[f"aw_{cname}"] = aw
        base[f"ab_{cname}"] = ab

    x = inputs[..., 0]                                   # [B, T, N]
    in_maps = []
    for c in range(NC_CORES):
        xs = x[c * BC:(c + 1) * BC]                      # [BC, T, N]
        xin = np.ascontiguousarray(
            np.transpose(xs, (1, 0, 2)).reshape(T, BC * N))
        xn = np.zeros((NCH, 128, BC * T), np.float32)
        for ch in range(NCH):
            blk = xs[:, :, ch * 128:(ch + 1) * 128]      # [BC, T, 128]
            xn[ch] = np.transpose(blk, (2, 0, 1)).reshape(128, BC * T)
        in_maps.append({**base, "xin": xin, "xn": np.ascontiguousarray(xn)})

    results = run(in_maps)
    out = np.zeros((B, NPRED, N, 1), np.float32)
    for c in range(NC_CORES):
        o = results[c]["out"].reshape(NPRED, BC, N)
        out[c * BC:(c + 1) * BC] = np.transpose(o, (1, 0, 2))[..., None]
    return out
